# revision 1
# baseline (speedup 1.0000x reference)
"""Trainium2 Bass kernel for moe_routing (nn_CITADEL_15118284882566).

Math: the reference collapses (qw >= 0; max rows always contain zeros from
non-matches, so negative branches never survive) to, per pair b:

    out[b] = sum_q qw[b,q] * relu( max_{l,kd} sims[b,q,l] * dw[b,l,kd]
                                   * [d_id[b,l,kd] == q_id[b,q]] )
             + dot(q_cls[b], d_cls[b])

Device strategy (data-parallel over B across 8 cores, 64 pairs/core,
16 groups of 4 pairs; partitions = 4 pairs x 32 queries):

1. DIFF2 = (d_id - q_id) + 2^-12 * dw via ONE K-stacked fp16 matmul per
   512-chunk (ids split hi/lo so every operand is fp16-exact; dw rows
   accumulate last so for matches the integer part cancels exactly and
   DIFF2 == fp16(dw) * 2^-12 EXACTLY; non-matches have |DIFF2| >= 0.999).
2. sims via 4 column-tiled matmuls (contraction over D=128 on partitions).
3. ACT copies DIFF2 * 2^20 -> fp16: matches become dw*2^8, non-matches
   saturate to +-inf.
4. (x*0)+x maps +-inf -> NaN (keeps dw); NaN-ignoring max-tree over kd;
   multiply by sims; NaN-ignoring reduce_max over l; relu * qw.
5. tok sums via one-hot matmul (scaled 2^-8); cls dots via elementwise
   mult + ones matmul; host adds the two tiny outputs.
"""
import sys

sys.path.insert(0, "/opt/trn_rl_repo")

import numpy as np

B, LQ, LD, KQ, KD, D = 512, 32, 512, 1, 5, 128
NCORES = 8
BPC = B // NCORES          # 64 pairs per core
NB = 4                     # pairs per group
G = BPC // NB              # 16 groups
P = 128
JD = KD * LD               # 2560
KSTACK = 14
EPS = 2.0 ** -12
SCALE = 2.0 ** 20

_CACHED = {}

# engine-assignment tuning (set by experiments)
DEFAULT_OPTS = dict(
    repeat=1,
    stt_nan=False,     # NaN-gen+add as one DVE scalar_tensor_tensor (1x rate)
    qrx_gpdma=True,    # rhx/qTx loads via SWDGE too
    gp_nan=False,      # NaN-gen TS on GPSIMD (else DVE)
    gp_nan_mod=0,      # if >0: groups with g % gp_nan_mod == 0 run nan-TS on DVE
    gp_prd=False,      # prd multiply on GPSIMD (else DVE)
    gp_tree=0,         # how many kd-tree max ops to run on GPSIMD (0-2)
    gp_msk4=False,     # compute kd4 mask-add on GPSIMD (DVE add covers kd0-3 only)
    fill_split=0,      # first N groups: split D2S/nz/add per-kd to cut fill latency
    scopy_act=False,   # S psum->sbuf copy on ACT (else DVE reads psum in prd)
    dT_gpdma=True,     # dT load via SWDGE/gpsimd (else sync)
    io_bufs=5,
    big_bufs=3,
    batch2=False,      # batch DVE mask/tree/reduce ops across pairs of groups
    act_nan_mod=0,     # if >0: groups with g % act_nan_mod == 0 run nan-gen on ACT
)


def _build_module(**kw):
    opts = dict(DEFAULT_OPTS)
    opts.update(kw)
    import concourse.bacc as bacc
    import concourse.mybir as mybir
    from concourse import tile

    f16 = mybir.dt.float16
    f32 = mybir.dt.float32
    Alu = mybir.AluOpType
    Act = mybir.ActivationFunctionType

    nc = bacc.Bacc("TRN2", target_bir_lowering=False, debug=False)

    # fused inputs
    dT_d = nc.dram_tensor("dT", [G, D, NB * LD], f16, kind="ExternalInput")
    qTx_d = nc.dram_tensor("qTx", [G, D, NB * LQ], f16, kind="ExternalInput")
    rhx_d = nc.dram_tensor("rhx", [G, KSTACK, JD + P], f16, kind="ExternalInput")
    epi_d = nc.dram_tensor("epi", [P, 2 * BPC + NB + 1 + G], f32, kind="ExternalInput")

    tok_d = nc.dram_tensor("tok", [NB, G], f32, kind="ExternalOutput")
    cls_d = nc.dram_tensor("cls", [1, BPC], f32, kind="ExternalOutput")

    with tile.TileContext(nc) as tc:
        with (
            tc.tile_pool(name="sb_io", bufs=opts["io_bufs"]) as sb_io,
            tc.tile_pool(name="sb_big", bufs=opts["big_bufs"]) as sb_big,
            tc.tile_pool(name="sb_wk", bufs=2) as sb_wk,
            tc.tile_pool(name="sb_res", bufs=1) as sb_res,
            tc.tile_pool(name="ps_diff", bufs=1, space="PSUM") as ps_diff,
            tc.tile_pool(name="ps_s", bufs=2, space="PSUM") as ps_s,
        ):
            res = sb_res.tile([P, G], f32)
            epi_t = sb_res.tile([P, 2 * BPC + NB + 1 + G], f32)
            nc.sync.dma_start(epi_t[:], epi_d[:])
            qw_all = epi_t[:, 2 * BPC + NB + 1:]

            for g_rep in range(opts["repeat"] * G):
                g = g_rep % G
                rhx_t = sb_io.tile([KSTACK, JD + P], f16, name="rhx_t")
                qTx_t = sb_io.tile([D, NB * LQ], f16, name="qTx_t")
                dT_t = sb_io.tile([D, NB * LD], f16, name="dT_t")
                qrx_eng = nc.gpsimd if opts["qrx_gpdma"] else nc.sync
                qrx_eng.dma_start(rhx_t[:], rhx_d[g, :, :])
                qrx_eng.dma_start(qTx_t[:], qTx_d[g, :, :])
                dma_eng = nc.gpsimd if opts["dT_gpdma"] else nc.sync
                dma_eng.dma_start(dT_t[:], dT_d[g, :, :])

                diff2 = ps_diff.tile([P, JD], f32, name="diff2")
                for k in range(KD):
                    nc.tensor.matmul(
                        diff2[:, k * LD:(k + 1) * LD],
                        rhx_t[:, JD:JD + P],
                        rhx_t[:, k * LD:(k + 1) * LD],
                        start=True, stop=True,
                    )
                s_ps = ps_s.tile([P, LD], f32, name="s_ps", tag="spool")
                for b in range(NB):
                    nc.tensor.matmul(
                        s_ps[b * LQ:(b + 1) * LQ, :],
                        qTx_t[:, b * LQ:(b + 1) * LQ],
                        dT_t[:, b * LD:(b + 1) * LD],
                        start=True, stop=True,
                        tile_position=(0, b * LQ),
                    )

                if opts["batch2"]:
                    # fill one half of the pair-wide tiles; DVE runs on odd g
                    half = g_rep % 2
                    if half == 0:
                        d2s2 = sb_big.tile([P, 2 * JD], f16, name="d2s2")
                        sps_prev = s_ps
                    nc.scalar.activation(d2s2[:, half * JD:(half + 1) * JD],
                                         diff2[:], Act.Copy, bias=0.0, scale=SCALE)
                    if half == 0:
                        continue
                    g0 = g - 1
                    msk2 = sb_big.tile([P, 2 * JD], f16, name="msk2")
                    nz2 = sb_big.tile([P, 2 * JD], f16, name="nz2")
                    nc.vector.tensor_scalar(nz2[:], d2s2[:], 0.0, None, Alu.mult)
                    nc.vector.tensor_tensor(msk2[:], nz2[:], d2s2[:], Alu.add)
                    m3 = msk2.rearrange("p (u j) -> p u j", u=2)
                    tA = sb_big.tile([P, 2 * LD], f16, name="tA")
                    tB = sb_big.tile([P, 2 * LD], f16, name="tB")
                    tA3 = tA.rearrange("p (u j) -> p u j", u=2)
                    tB3 = tB.rearrange("p (u j) -> p u j", u=2)
                    def ksl(k):
                        return m3[:, :, k * LD:(k + 1) * LD]
                    nc.vector.tensor_tensor(tA3[:], ksl(0), ksl(1), Alu.max)
                    nc.vector.tensor_tensor(tB3[:], ksl(2), ksl(3), Alu.max)
                    nc.vector.tensor_tensor(tA3[:], tA3[:], tB3[:], Alu.max)
                    nc.vector.tensor_tensor(tA3[:], tA3[:], ksl(4), Alu.max)
                    prd2 = sb_big.tile([P, 2 * LD], f16, name="prd2")
                    nc.vector.tensor_tensor(prd2[:, 0:LD], sps_prev[:], tA[:, 0:LD], Alu.mult)
                    nc.vector.tensor_tensor(prd2[:, LD:], s_ps[:], tA[:, LD:], Alu.mult)
                    mx2 = sb_big.tile([P, 2], f32, name="mx2")
                    nc.vector.reduce_max(
                        mx2[:], prd2.rearrange("p (u j) -> p u j", u=2)[:],
                        axis=mybir.AxisListType.X)
                    rl2 = sb_big.tile([P, 2], f32, name="rl2")
                    nc.vector.tensor_scalar(rl2[:], mx2[:], 0.0, None, Alu.max)
                    nc.vector.tensor_tensor(res[:, g0:g0 + 2], rl2[:],
                                            qw_all[:, g0:g0 + 2], Alu.mult)
                    continue

                d2s = sb_big.tile([P, JD], f16, name="d2s")
                if g_rep < opts["fill_split"]:
                    for k in range(KD):
                        sl = slice(k * LD, (k + 1) * LD)
                        nc.scalar.activation(d2s[:, sl], diff2[:, sl], Act.Copy,
                                             bias=0.0, scale=SCALE)
                else:
                    nc.scalar.activation(d2s[:], diff2[:], Act.Copy, bias=0.0, scale=SCALE)

                msk = sb_big.tile([P, JD], f16, name="msk")
                nz = sb_big.tile([P, JD], f16, name="nz")
                nan_eng = nc.gpsimd if opts["gp_nan"] else nc.vector
                if opts["gp_nan_mod"] and g_rep % opts["gp_nan_mod"] == 0:
                    nan_eng = nc.vector
                act_nan = opts["act_nan_mod"] and g_rep % opts["act_nan_mod"] == 0
                if opts["stt_nan"]:
                    nc.vector.scalar_tensor_tensor(
                        msk[:], d2s[:], 0.0, d2s[:], Alu.mult, Alu.add)
                elif g_rep < opts["fill_split"]:
                    for k in range(KD):
                        sl = slice(k * LD, (k + 1) * LD)
                        nan_eng.tensor_scalar(nz[:, sl], d2s[:, sl], 0.0, None, Alu.mult)
                        nc.vector.tensor_tensor(msk[:, sl], nz[:, sl], d2s[:, sl], Alu.add)
                elif opts["gp_msk4"]:
                    nan_eng.tensor_scalar(nz[:], d2s[:], 0.0, None, Alu.mult)
                    nc.vector.tensor_tensor(
                        msk[:, 0:4 * LD], nz[:, 0:4 * LD], d2s[:, 0:4 * LD], Alu.add)
                    nc.gpsimd.tensor_tensor(
                        msk[:, 4 * LD:], nz[:, 4 * LD:], d2s[:, 4 * LD:], Alu.add)
                else:
                    if act_nan:
                        nc.scalar.activation(nz[:], d2s[:], Act.Copy, bias=0.0, scale=0.0)
                    else:
                        nan_eng.tensor_scalar(nz[:], d2s[:], 0.0, None, Alu.mult)
                    nc.vector.tensor_tensor(msk[:], nz[:], d2s[:], Alu.add)

                t01 = sb_wk.tile([P, LD], f16, name="t01")
                t23 = sb_wk.tile([P, LD], f16, name="t23")
                e1 = nc.gpsimd if opts["gp_tree"] >= 1 else nc.vector
                e2 = nc.gpsimd if opts["gp_tree"] >= 2 else nc.vector
                nc.vector.tensor_tensor(t01[:], msk[:, 0:LD], msk[:, LD:2 * LD], Alu.max)
                e1.tensor_tensor(t23[:], msk[:, 2 * LD:3 * LD], msk[:, 3 * LD:4 * LD], Alu.max)
                e2.tensor_tensor(t01[:], t01[:], t23[:], Alu.max)
                dmx = sb_wk.tile([P, LD], f16, name="dmx")
                nc.vector.tensor_tensor(dmx[:], t01[:], msk[:, 4 * LD:5 * LD], Alu.max)

                prd = sb_wk.tile([P, LD], f16, name="prd")
                if opts["scopy_act"]:
                    scp = sb_wk.tile([P, LD], f16, name="scp")
                    nc.scalar.activation(scp[:], s_ps[:], Act.Copy, bias=0.0, scale=1.0)
                    prd_eng = nc.gpsimd if opts["gp_prd"] else nc.vector
                    prd_eng.tensor_tensor(prd[:], scp[:], dmx[:], Alu.mult)
                else:
                    nc.vector.tensor_tensor(prd[:], s_ps[:], dmx[:], Alu.mult)
                mx = sb_wk.tile([P, 1], f32, name="mx")
                nc.vector.reduce_max(mx[:], prd[:], axis=mybir.AxisListType.X)
                # res[:, g] = max(mx, 0) * qw   (carries the 2^8 factor; the
                # epilogue one-hot matmul is scaled by 2^-8 to undo it)
                nc.vector.tensor_scalar(
                    res[:, g:g + 1], mx[:], 0.0, qw_all[:, g:g + 1],
                    Alu.max, Alu.mult,
                )

            # ---- epilogue: tok colsums + cls dots ----
            qcT_t = epi_t[:, 0:BPC]
            dcT_t = epi_t[:, BPC:2 * BPC]
            e4s_t = epi_t[:, 2 * BPC:2 * BPC + NB]
            ones_t = epi_t[:, 2 * BPC + NB:2 * BPC + NB + 1]

            cp = sb_res.tile([D, BPC], f32)
            nc.vector.tensor_tensor(cp[:], qcT_t, dcT_t, Alu.mult)

            tok_ps = ps_s.tile([NB, G], f32, name="tok_ps", tag="spool")
            nc.tensor.matmul(tok_ps[:], e4s_t, res[:], start=True, stop=True)
            cls_ps = ps_s.tile([1, BPC], f32, name="cls_ps", tag="spool")
            nc.tensor.matmul(cls_ps[:], ones_t, cp[:], start=True, stop=True)

            tok_sb = sb_res.tile([NB, G], f32)
            cls_sb = sb_res.tile([1, BPC], f32)
            nc.vector.tensor_copy(tok_sb[:], tok_ps[:])
            nc.vector.tensor_copy(cls_sb[:], cls_ps[:])
            nc.gpsimd.dma_start(tok_d[:], tok_sb[:])
            nc.gpsimd.dma_start(cls_d[:], cls_sb[:])

    nc.compile()
    return nc


def _prep_core_inputs(c, q_repr, q_w, q_ids, q_cls, d_repr, d_w, d_ids, d_cls):
    """Pure layout/packing for one core's 64 pairs."""
    s = slice(c * BPC, (c + 1) * BPC)
    qr = q_repr[s]          # [64, 32, 128] f32
    qw = q_w[s, :, 0]       # [64, 32]
    qi = q_ids[s, :, 0]     # [64, 32] int64
    qc = q_cls[s]           # [64, 128]
    dr = d_repr[s]          # [64, 512, 128]
    dw = d_w[s]             # [64, 512, 5]
    di = d_ids[s]           # [64, 512, 5]
    dc = d_cls[s]           # [64, 128]

    # dT: [G, D, NB*LD] = per group, 4 batches' transposed reprs side by side
    dT = np.ascontiguousarray(
        dr.reshape(G, NB, LD, D).transpose(0, 3, 1, 2).reshape(G, D, NB * LD)
    ).astype(np.float16)

    qTx = np.ascontiguousarray(
        qr.reshape(G, NB, LQ, D).transpose(0, 3, 1, 2).reshape(G, D, NB * LQ)
    ).astype(np.float16)
    qww = qw.reshape(G, NB * LQ)  # partition p = 32*b + q

    q_hi = (qi >> 8).astype(np.float32)
    q_lo = (qi & 255).astype(np.float32)
    d_hi = (di >> 8).astype(np.float32)
    d_lo = (di & 255).astype(np.float32)
    dw16 = dw.astype(np.float16).astype(np.float32)

    E = np.zeros((NB, P), np.float32)
    for b in range(NB):
        E[b, b * LQ:(b + 1) * LQ] = 1.0

    # rhx: [G, KSTACK, JD + P]: cols [0, JD) = rhs, cols [JD, JD+P) = lhsT
    rhx = np.zeros((G, KSTACK, JD + P), np.float32)
    rhx[:, 0:4, :JD] = d_hi.reshape(G, NB, LD, KD).transpose(0, 1, 3, 2).reshape(G, NB, JD)
    rhx[:, 4:8, :JD] = d_lo.reshape(G, NB, LD, KD).transpose(0, 1, 3, 2).reshape(G, NB, JD)
    rhx[:, 8, :JD] = 256.0
    rhx[:, 9, :JD] = 1.0
    rhx[:, 10:14, :JD] = dw16.reshape(G, NB, LD, KD).transpose(0, 1, 3, 2).reshape(G, NB, JD)
    rhx[:, 0:4, JD:] = 256.0 * E
    rhx[:, 4:8, JD:] = E
    rhx[:, 8, JD:] = -q_hi.reshape(G, P)
    rhx[:, 9, JD:] = -q_lo.reshape(G, P)
    rhx[:, 10:14, JD:] = EPS * E

    epi = np.zeros((P, 2 * BPC + NB + 1 + G), np.float32)
    epi[:, 0:BPC] = qc.T
    epi[:, BPC:2 * BPC] = dc.T
    for b in range(NB):
        epi[b * LQ:(b + 1) * LQ, 2 * BPC + b] = 2.0 ** -8
    epi[:, 2 * BPC + NB] = 1.0
    epi[:, 2 * BPC + NB + 1:] = qww.T

    return {
        "dT": dT,
        "qTx": qTx,
        "rhx": rhx.astype(np.float16),
        "epi": epi,
    }


def kernel(q_expert_repr, q_expert_weights, q_expert_ids, q_cls_repr,
           d_expert_repr, d_expert_weights, d_expert_ids, d_cls_repr):
    from concourse.bass_utils import run_bass_kernel_spmd

    q_repr = np.asarray(q_expert_repr, np.float32)
    q_w = np.asarray(q_expert_weights, np.float32)
    q_ids = np.asarray(q_expert_ids, np.int64)
    q_cls = np.asarray(q_cls_repr, np.float32)
    d_repr = np.asarray(d_expert_repr, np.float32)
    d_w = np.asarray(d_expert_weights, np.float32)
    d_ids = np.asarray(d_expert_ids, np.int64)
    d_cls = np.asarray(d_cls_repr, np.float32)

    if "nc" not in _CACHED:
        _CACHED["nc"] = _build_module()
    nc = _CACHED["nc"]

    in_maps = [
        _prep_core_inputs(c, q_repr, q_w, q_ids, q_cls, d_repr, d_w, d_ids, d_cls)
        for c in range(NCORES)
    ]
    rr = run_bass_kernel_spmd(nc, in_maps, core_ids=list(range(NCORES)))

    out = np.zeros((B,), np.float32)
    for c in range(NCORES):
        tok = rr.results[c]["tok"]          # [NB, G]
        cls = rr.results[c]["cls"][0]       # [BPC]
        out[c * BPC:(c + 1) * BPC] = tok.T.reshape(-1) + cls
    return out


if __name__ == "__main__":
    rng = np.random.default_rng(0)
    ins = {
        "q_expert_repr": rng.standard_normal((B, LQ, D)).astype(np.float32),
        "q_expert_weights": rng.random((B, LQ, KQ)).astype(np.float32),
        "q_expert_ids": rng.integers(0, 30522, (B, LQ, KQ)).astype(np.int64),
        "q_cls_repr": rng.standard_normal((B, D)).astype(np.float32),
        "d_expert_repr": rng.standard_normal((B, LD, D)).astype(np.float32),
        "d_expert_weights": rng.random((B, LD, KD)).astype(np.float32),
        "d_expert_ids": rng.integers(0, 30522, (B, LD, KD)).astype(np.int64),
        "d_cls_repr": rng.standard_normal((B, D)).astype(np.float32),
    }
    out = kernel(**ins)
    print("kernel out[:8]:", out[:8])



# revision 4
# speedup vs baseline: 1.1901x; 1.1901x over previous
"""Trainium2 Bass kernel for moe_routing (nn_CITADEL_15118284882566).

Math: the reference collapses (qw >= 0; the max rows always contain zeros
from non-matches, so negative branches never survive) to, per pair b:

    out[b] = sum_q qw[b,q] * relu( max_{l,kd} sims[b,q,l] * dw[b,l,kd]
                                   * [d_id[b,l,kd] == q_id[b,q]] )
             + dot(q_cls[b], d_cls[b])

Device strategy (data-parallel over B across 8 cores, 64 pairs/core,
16 groups of 4 pairs; partitions = 4 pairs x 32 queries):

1. DIFF2 = -(d_id - q_id)^2 + 2^-12 * dw via one 26-row-stacked f16 matmul
   per column chunk. Ids are split into three 5-bit chunks (a,b,c <= 31) so
   -(d-q)^2 = -(da-qa)^2-(db-qb)^2-(dc-qc)^2 expands into bilinear rows
   whose operands are all fp16-exact integers; PSUM f32 accumulation is
   exact, dw rows accumulate last. Matches give DIFF2 = 2^-12*fp16(dw)
   EXACTLY; non-matches give DIFF2 <= -1 + 2^-12 (strictly negative).
2. ACT relu-copies DIFF2 * 2^20 -> f16: matches become dw*2^8, non-matches
   become 0. No NaN/inf masking needed anywhere.
3. sims via 4 column-tiled matmuls (contraction over D=128 on partitions).
4. max-tree over kd (DVE/GPSIMD); multiply by sims; reduce_max over l;
   relu * qw.
5. tok sums via one-hot matmul (scaled 2^-8); cls dots via elementwise
   mult + ones matmul; host adds the two tiny outputs.

PSUM: diff2 is computed in two l-chunks of [128, 5*256] f32 (3 banks each,
double-buffered = 6 banks) + sims [128,512] (1 bank x2) = 8 banks, which
decouples the PE->ACT pipeline at half-group granularity.
"""
import sys

sys.path.insert(0, "/opt/trn_rl_repo")

import numpy as np

B, LQ, LD, KQ, KD, D = 512, 32, 512, 1, 5, 128
NCORES = 8
BPC = B // NCORES          # 64 pairs per core
NB = 4                     # pairs per group
G = BPC // NB              # 16 groups
P = 128
JD = KD * LD               # 2560
LC = 2                     # l-chunks per group
LDC = LD // LC             # 256 l per chunk
JC = KD * LDC              # 1280 cols per chunk
KSTACK = 26
EPS = 2.0 ** -12
SCALE = 2.0 ** 20

_CACHED = {}

# engine-assignment tuning
DEFAULT_OPTS = dict(
    gp_tree=0,        # how many kd-tree max ops to run on GPSIMD (0-2);
                      # NOTE: Pool TENSOR_TENSOR fails the V3 ISA engine
                      # check in this toolchain — keep 0.
    s_sb="act",       # sims PSUM->SBUF copy: "act" | "dve" | "none" (prd reads PSUM)
    d2s_dve_lc=0,     # how many of the 2 d2s chunks to process on DVE (ts mult+max)
    io_bufs=3,
    d2s_bufs=2,
    dtq_eng="sync",   # dTq DMA engine: sync | scalar | gpsimd
    rhx_eng="scalar", # rhx DMA engine
)


def _build_module(**kw):
    opts = dict(DEFAULT_OPTS)
    opts.update(kw)
    import concourse.bacc as bacc
    import concourse.mybir as mybir
    from concourse import tile

    f16 = mybir.dt.float16
    f32 = mybir.dt.float32
    Alu = mybir.AluOpType
    Act = mybir.ActivationFunctionType

    nc = bacc.Bacc("TRN2", target_bir_lowering=False, debug=False)

    def eng(name):
        return {"sync": nc.sync, "scalar": nc.scalar, "gpsimd": nc.gpsimd}[name]

    # fused inputs
    dtq_d = nc.dram_tensor("dtq", [G, D, NB * LD + NB * LQ], f16, kind="ExternalInput")
    rhx_d = nc.dram_tensor("rhx", [G, KSTACK, JD + P], f16, kind="ExternalInput")
    epi_d = nc.dram_tensor("epi", [P, 2 * BPC + NB + 1 + G], f32, kind="ExternalInput")

    tok_d = nc.dram_tensor("tok", [NB, G], f32, kind="ExternalOutput")
    cls_d = nc.dram_tensor("cls", [1, BPC], f32, kind="ExternalOutput")

    with tile.TileContext(nc) as tc:
        with (
            tc.tile_pool(name="sb_io", bufs=opts["io_bufs"]) as sb_io,
            tc.tile_pool(name="sb_big", bufs=opts["d2s_bufs"]) as sb_big,
            tc.tile_pool(name="sb_wk", bufs=2) as sb_wk,
            tc.tile_pool(name="sb_res", bufs=1) as sb_res,
            tc.tile_pool(name="ps_diff", bufs=2, space="PSUM") as ps_diff,
            tc.tile_pool(name="ps_s", bufs=2, space="PSUM") as ps_s,
        ):
            res = sb_res.tile([P, G], f32)
            epi_t = sb_res.tile([P, 2 * BPC + NB + 1 + G], f32)
            nc.sync.dma_start(epi_t[:], epi_d[:])
            qw_all = epi_t[:, 2 * BPC + NB + 1:]

            for g in range(G):
                dtq_t = sb_io.tile([D, NB * LD + NB * LQ], f16, name="dtq_t")
                rhx_t = sb_io.tile([KSTACK, JD + P], f16, name="rhx_t")
                eng(opts["dtq_eng"]).dma_start(dtq_t[:], dtq_d[g, :, :])
                eng(opts["rhx_eng"]).dma_start(rhx_t[:], rhx_d[g, :, :])

                d2s = sb_big.tile([P, JD], f16, name="d2s")
                lhsT = rhx_t[:, JD:JD + P]
                for lc in range(LC):
                    dfc = ps_diff.tile([P, JC], f32, name="dfc")
                    for k in range(KD):
                        nc.tensor.matmul(
                            dfc[:, k * LDC:(k + 1) * LDC],
                            lhsT,
                            rhx_t[:, lc * JC + k * LDC: lc * JC + (k + 1) * LDC],
                            start=True, stop=True,
                        )
                    if lc < LC - opts["d2s_dve_lc"]:
                        nc.scalar.activation(d2s[:, lc * JC:(lc + 1) * JC], dfc[:],
                                             Act.Relu, bias=0.0, scale=SCALE)
                    else:
                        nc.vector.tensor_scalar(d2s[:, lc * JC:(lc + 1) * JC], dfc[:],
                                                SCALE, 0.0, Alu.mult, Alu.max)

                s_ps = ps_s.tile([P, LD], f32, name="s_ps", tag="spool")
                for b in range(NB):
                    nc.tensor.matmul(
                        s_ps[b * LQ:(b + 1) * LQ, :],
                        dtq_t[:, NB * LD + b * LQ: NB * LD + (b + 1) * LQ],
                        dtq_t[:, b * LD:(b + 1) * LD],
                        start=True, stop=True,
                        tile_position=(0, b * LQ),
                    )

                # kd max-tree over d2s viewed as [p, lc, kd, LDC]
                m4 = d2s.rearrange("p (u k j) -> p u k j", u=LC, k=KD)

                def ksl(k):
                    return m4[:, :, k, :]

                t01 = sb_wk.tile([P, LD], f16, name="t01")
                t23 = sb_wk.tile([P, LD], f16, name="t23")
                t01r = t01.rearrange("p (u j) -> p u j", u=LC)
                t23r = t23.rearrange("p (u j) -> p u j", u=LC)
                e1 = nc.gpsimd if opts["gp_tree"] >= 1 else nc.vector
                e2 = nc.gpsimd if opts["gp_tree"] >= 2 else nc.vector
                nc.vector.tensor_tensor(t01r[:], ksl(0), ksl(1), Alu.max)
                e1.tensor_tensor(t23r[:], ksl(2), ksl(3), Alu.max)
                e2.tensor_tensor(t01r[:], t01r[:], t23r[:], Alu.max)
                dmx = sb_wk.tile([P, LD], f16, name="dmx")
                dmxr = dmx.rearrange("p (u j) -> p u j", u=LC)
                nc.vector.tensor_tensor(dmxr[:], t01r[:], ksl(4), Alu.max)

                prd = sb_wk.tile([P, LD], f16, name="prd")
                if opts["s_sb"] == "none":
                    nc.vector.tensor_tensor(prd[:], s_ps[:], dmx[:], Alu.mult)
                else:
                    s_sb = sb_wk.tile([P, LD], f16, name="s_sb")
                    if opts["s_sb"] == "act":
                        nc.scalar.activation(s_sb[:], s_ps[:], Act.Copy,
                                             bias=0.0, scale=1.0)
                    else:
                        nc.vector.tensor_copy(s_sb[:], s_ps[:])
                    nc.vector.tensor_tensor(prd[:], s_sb[:], dmx[:], Alu.mult)
                mx = sb_wk.tile([P, 1], f32, name="mx")
                nc.vector.reduce_max(mx[:], prd[:], axis=mybir.AxisListType.X)
                # res[:, g] = max(mx, 0) * qw   (carries the 2^8 factor; the
                # epilogue one-hot matmul is scaled by 2^-8 to undo it)
                nc.vector.tensor_scalar(
                    res[:, g:g + 1], mx[:], 0.0, qw_all[:, g:g + 1],
                    Alu.max, Alu.mult,
                )

            # ---- epilogue: tok colsums + cls dots ----
            qcT_t = epi_t[:, 0:BPC]
            dcT_t = epi_t[:, BPC:2 * BPC]
            e4s_t = epi_t[:, 2 * BPC:2 * BPC + NB]
            ones_t = epi_t[:, 2 * BPC + NB:2 * BPC + NB + 1]

            cp = sb_res.tile([D, BPC], f32)
            nc.vector.tensor_tensor(cp[:], qcT_t, dcT_t, Alu.mult)

            tok_ps = ps_s.tile([NB, G], f32, name="tok_ps", tag="spool")
            nc.tensor.matmul(tok_ps[:], e4s_t, res[:], start=True, stop=True)
            cls_ps = ps_s.tile([1, BPC], f32, name="cls_ps", tag="spool")
            nc.tensor.matmul(cls_ps[:], ones_t, cp[:], start=True, stop=True)

            tok_sb = sb_res.tile([NB, G], f32)
            cls_sb = sb_res.tile([1, BPC], f32)
            nc.vector.tensor_copy(tok_sb[:], tok_ps[:])
            nc.vector.tensor_copy(cls_sb[:], cls_ps[:])
            nc.sync.dma_start(tok_d[:], tok_sb[:])
            nc.sync.dma_start(cls_d[:], cls_sb[:])

    nc.compile()
    return nc


def _prep_core_inputs(c, q_repr, q_w, q_ids, q_cls, d_repr, d_w, d_ids, d_cls):
    """Pure layout/packing for one core's 64 pairs."""
    s = slice(c * BPC, (c + 1) * BPC)
    qr = q_repr[s]          # [64, 32, 128] f32
    qw = q_w[s, :, 0]       # [64, 32]
    qi = q_ids[s, :, 0]     # [64, 32] int64
    qc = q_cls[s]           # [64, 128]
    dr = d_repr[s]          # [64, 512, 128]
    dw = d_w[s]             # [64, 512, 5]
    di = d_ids[s]           # [64, 512, 5]
    dc = d_cls[s]           # [64, 128]

    # dtq: [G, D, NB*LD + NB*LQ]: dT cols then qTx cols
    dtq = np.empty((G, D, NB * LD + NB * LQ), np.float16)
    dtq[:, :, :NB * LD] = (
        dr.reshape(G, NB, LD, D).transpose(0, 3, 1, 2).reshape(G, D, NB * LD)
    ).astype(np.float16)
    dtq[:, :, NB * LD:] = (
        qr.reshape(G, NB, LQ, D).transpose(0, 3, 1, 2).reshape(G, D, NB * LQ)
    ).astype(np.float16)
    qww = qw.reshape(G, NB * LQ)  # partition p = 32*b + q

    # 5-bit id chunks (ids < 2^15)
    qa = (qi >> 10).astype(np.float32)
    qb = ((qi >> 5) & 31).astype(np.float32)
    qcq = (qi & 31).astype(np.float32)
    da = (di >> 10).astype(np.float32)
    db = ((di >> 5) & 31).astype(np.float32)
    dcc = (di & 31).astype(np.float32)
    dw16 = dw.astype(np.float16).astype(np.float32)

    E = np.zeros((NB, P), np.float32)
    for b in range(NB):
        E[b, b * LQ:(b + 1) * LQ] = 1.0

    def dcols(x):
        # [G*NB, LD, KD] batch values -> [G, NB, JD] in l-chunk-major
        # column order: j = lc*JC + kd*LDC + (l - lc*LDC)
        return (x.reshape(G, NB, LC, LDC, KD)
                 .transpose(0, 1, 2, 4, 3).reshape(G, NB, JD))

    # rhx: [G, KSTACK, JD + P]: cols [0, JD) = rhs, cols [JD, JD+P) = lhsT
    rhx = np.zeros((G, KSTACK, JD + P), np.float32)
    rhx[:, 0:4, :JD] = dcols(da * da + db * db)
    rhx[:, 4:8, :JD] = dcols(dcc * dcc)
    rhx[:, 8:12, :JD] = dcols(da)
    rhx[:, 12:16, :JD] = dcols(db)
    rhx[:, 16:20, :JD] = dcols(dcc)
    rhx[:, 20, :JD] = 1.0
    rhx[:, 21, :JD] = 1.0
    rhx[:, 22:26, :JD] = dcols(dw16)

    qar = qa.reshape(G, P)
    qbr = qb.reshape(G, P)
    qcr = qcq.reshape(G, P)
    rhx[:, 0:4, JD:] = -E
    rhx[:, 4:8, JD:] = -E
    rhx[:, 8:12, JD:] = (2.0 * qar)[:, None, :] * E
    rhx[:, 12:16, JD:] = (2.0 * qbr)[:, None, :] * E
    rhx[:, 16:20, JD:] = (2.0 * qcr)[:, None, :] * E
    rhx[:, 20, JD:] = -(qar * qar + qbr * qbr)
    rhx[:, 21, JD:] = -(qcr * qcr)
    rhx[:, 22:26, JD:] = EPS * E

    epi = np.zeros((P, 2 * BPC + NB + 1 + G), np.float32)
    epi[:, 0:BPC] = qc.T
    epi[:, BPC:2 * BPC] = dc.T
    for b in range(NB):
        epi[b * LQ:(b + 1) * LQ, 2 * BPC + b] = 2.0 ** -8
    epi[:, 2 * BPC + NB] = 1.0
    epi[:, 2 * BPC + NB + 1:] = qww.T

    return {
        "dtq": dtq,
        "rhx": rhx.astype(np.float16),
        "epi": epi,
    }


def kernel(q_expert_repr, q_expert_weights, q_expert_ids, q_cls_repr,
           d_expert_repr, d_expert_weights, d_expert_ids, d_cls_repr):
    from concourse.bass_utils import run_bass_kernel_spmd

    q_repr = np.asarray(q_expert_repr, np.float32)
    q_w = np.asarray(q_expert_weights, np.float32)
    q_ids = np.asarray(q_expert_ids, np.int64)
    q_cls = np.asarray(q_cls_repr, np.float32)
    d_repr = np.asarray(d_expert_repr, np.float32)
    d_w = np.asarray(d_expert_weights, np.float32)
    d_ids = np.asarray(d_expert_ids, np.int64)
    d_cls = np.asarray(d_cls_repr, np.float32)

    if "nc" not in _CACHED:
        _CACHED["nc"] = _build_module()
    nc = _CACHED["nc"]

    in_maps = [
        _prep_core_inputs(c, q_repr, q_w, q_ids, q_cls, d_repr, d_w, d_ids, d_cls)
        for c in range(NCORES)
    ]
    rr = run_bass_kernel_spmd(nc, in_maps, core_ids=list(range(NCORES)))

    out = np.zeros((B,), np.float32)
    for c in range(NCORES):
        tok = rr.results[c]["tok"]          # [NB, G]
        cls = rr.results[c]["cls"][0]       # [BPC]
        out[c * BPC:(c + 1) * BPC] = tok.T.reshape(-1) + cls
    return out


if __name__ == "__main__":
    rng = np.random.default_rng(0)
    ins = {
        "q_expert_repr": rng.standard_normal((B, LQ, D)).astype(np.float32),
        "q_expert_weights": rng.random((B, LQ, KQ)).astype(np.float32),
        "q_expert_ids": rng.integers(0, 30522, (B, LQ, KQ)).astype(np.int64),
        "q_cls_repr": rng.standard_normal((B, D)).astype(np.float32),
        "d_expert_repr": rng.standard_normal((B, LD, D)).astype(np.float32),
        "d_expert_weights": rng.random((B, LD, KD)).astype(np.float32),
        "d_expert_ids": rng.integers(0, 30522, (B, LD, KD)).astype(np.int64),
        "d_cls_repr": rng.standard_normal((B, D)).astype(np.float32),
    }
    out = kernel(**ins)
    print("kernel out[:8]:", out[:8])


# revision 24
# speedup vs baseline: 1.5013x; 1.2615x over previous
"""Trainium2 Bass kernel for moe_routing (nn_CITADEL_15118284882566).

Math: the reference collapses (qw >= 0; the max rows always contain zeros
from non-matches, so negative branches never survive) to, per pair b:

    out[b] = sum_q qw[b,q] * relu( max_{l,kd} sims[b,q,l] * dw[b,l,kd]
                                   * [d_id[b,l,kd] == q_id[b,q]] )
             + dot(q_cls[b], d_cls[b])

Device strategy (data-parallel over B across 8 cores, 64 pairs/core,
16 groups of 4 pairs; partitions = 4 pairs x 32 queries):

1. DIFF2 = -(d_id - q_id)^2 + 2^-12 * dw via 26-row-stacked f16 matmuls
   (two l-chunks of [128, 5*256] f32 PSUM, double buffered). Ids are split
   into three 5-bit chunks (a,b,c <= 31) so -(d-q)^2 expands into bilinear
   rows whose operands are all fp16-exact integers; PSUM f32 accumulation
   is exact, dw rows accumulate last. Matches give DIFF2 = 2^-12*fp16(dw)
   EXACTLY; non-matches give DIFF2 <= -1 + 2^-12 (strictly negative).
2. ACT relu-copies DIFF2 * 2^20 -> f16 (kd-major d2s layout): matches
   become dw*2^8, non-matches 0. No NaN/inf masking needed anywhere.
3. sims via 4 column-tiled matmuls (contraction over D=128 on partitions);
   qw is pre-folded into qT on the host.
4. kd max-tree (3 DVE ops), prd = sims*dmx, reduce_max over l.
5. Single merged epilogue matmul [e4s|ones].T @ [res|cp] -> tok sums
   (scaled 2^-8) and cls dots in one [5, G+BPC] output, one DMA.

PSUM budget: dfc [128,1280] x2 bufs = 6 banks + sims [128,512] x2 = 8.
"""
import sys

sys.path.insert(0, "/opt/trn_rl_repo")

import numpy as np

B, LQ, LD, KQ, KD, D = 512, 32, 512, 1, 5, 128
NCORES = 8
BPC = B // NCORES          # 64 pairs per core
NB = 4                     # pairs per group
G = BPC // NB              # 16 groups
P = 128
JD = KD * LD               # 2560
LDC = LD // 2              # 256 l per l-chunk
JC = KD * LDC              # 1280 cols per chunk
KSTACK = 26
EPS = 2.0 ** -12
SCALE = 2.0 ** 20

_CACHED = {}

# engine-assignment tuning
# NOTE: Pool (GPSIMD) TENSOR_TENSOR fails the V3 ISA engine check in this
# toolchain — GPSIMD can only do DMA triggering / tensor_copy. Concurrent
# PE row-group matmuls writing the same PSUM bank hang the device.
DEFAULT_OPTS = dict(
    dtq_bufs=3,
    rhx_bufs=4,
    d2s_bufs=2,
    dtq_eng="sync",   # dTq DMA engine: sync | scalar | gpsimd
    rhx_eng="gpsimd", # rhx DMA engine
    rowpack=True,     # 2-way PE row-group packing: row-block = l-chunk, so
                      # concurrent streams write disjoint PSUM tiles/banks
)


def _build_module(**kw):
    opts = dict(DEFAULT_OPTS)
    opts.update(kw)
    import concourse.bacc as bacc
    import concourse.mybir as mybir
    from concourse import tile

    f16 = mybir.dt.float16
    f32 = mybir.dt.float32
    Alu = mybir.AluOpType
    Act = mybir.ActivationFunctionType

    nc = bacc.Bacc("TRN2", target_bir_lowering=False, debug=False)

    def eng(name):
        return {"sync": nc.sync, "scalar": nc.scalar, "gpsimd": nc.gpsimd}[name]

    # fused inputs
    dtq_d = nc.dram_tensor("dtq", [G, D, NB * LD + NB * LQ], f16, kind="ExternalInput")
    rhx_d = nc.dram_tensor("rhx", [G, 2, KSTACK, 128 + JC], f16, kind="ExternalInput")
    epi_d = nc.dram_tensor("epi", [P, 2 * BPC + NB + 1], f32, kind="ExternalInput")

    out_d = nc.dram_tensor("out", [NB + 1, G + BPC], f32, kind="ExternalOutput")

    with tile.TileContext(nc) as tc:
        with (
            tc.tile_pool(name="sb_dtq", bufs=opts["dtq_bufs"]) as sb_dtq,
            tc.tile_pool(name="sb_rhx", bufs=opts["rhx_bufs"]) as sb_rhx,
            tc.tile_pool(name="sb_big", bufs=opts["d2s_bufs"]) as sb_big,
            tc.tile_pool(name="sb_wk", bufs=2) as sb_wk,
            tc.tile_pool(name="sb_res", bufs=1) as sb_res,
            tc.tile_pool(name="ps_diff", bufs=2, space="PSUM") as ps_diff,
            tc.tile_pool(name="ps_s", bufs=2, space="PSUM") as ps_s,
        ):
            # rescp: cols [0, G) = res (per-group relu'd maxes; qw is folded
            # into sims), cols [G, G+BPC) = cp (cls elementwise prods)
            rescp = sb_res.tile([P, G + BPC], f32)
            mxall = sb_res.tile([P, G], f32)
            epi_t = sb_res.tile([P, 2 * BPC + NB + 1], f32)
            nc.sync.dma_start(epi_t[:], epi_d[:])
            # cls products, ready as soon as epi lands
            nc.vector.tensor_tensor(rescp[:, G:], epi_t[:, 0:BPC],
                                    epi_t[:, BPC:2 * BPC], Alu.mult)

            for g in range(G):
                dtq_t = sb_dtq.tile([D, NB * LD + NB * LQ], f16, name="dtq_t")
                rhx_t = sb_rhx.tile([58, 128 + JC], f16, name="rhx_t")
                eng(opts["rhx_eng"]).dma_start(rhx_t[0:KSTACK, :], rhx_d[g, 0])
                eng(opts["rhx_eng"]).dma_start(rhx_t[32:32 + KSTACK, :],
                                               rhx_d[g, 1])
                eng(opts["dtq_eng"]).dma_start(dtq_t[:], dtq_d[g, :, :])

                # d2s column layout is kd-major over full l: col = k*LD + l
                # (l = lc*LDC + j), so every tree slice is 2D-contiguous.
                d2s = sb_big.tile([P, JD], f16, name="d2s")
                d2s_k = d2s.rearrange("p (k j) -> p k j", k=KD)
                # row-block = l-chunk: the two dfc pool slots are disjoint
                # PSUM bank sets, so the two PE row-group streams never
                # write the same bank (same-bank interleave hangs the HW).
                dfc0 = ps_diff.tile([P, JC], f32, name="dfc")
                dfc1 = ps_diff.tile([P, JC], f32, name="dfc")
                for k in range(KD):
                    for lc, dfc in ((0, dfc0), (1, dfc1)):
                        base = 32 * lc if opts["rowpack"] else 0
                        nc.tensor.matmul(
                            dfc[:, k * LDC:(k + 1) * LDC],
                            rhx_t[base:base + KSTACK, 0:P],
                            rhx_t[base:base + KSTACK,
                                  128 + k * LDC: 128 + (k + 1) * LDC],
                            start=True, stop=True,
                        )
                for lc, dfc in ((0, dfc0), (1, dfc1)):
                    nc.scalar.activation(
                        d2s_k[:, :, lc * LDC:(lc + 1) * LDC],
                        dfc.rearrange("p (k j) -> p k j", k=KD),
                        Act.Relu, bias=0.0, scale=SCALE)

                s_ps = ps_s.tile([P, LD], f32, name="s_ps", tag="spool")
                for b in range(NB):
                    nc.tensor.matmul(
                        s_ps[b * LQ:(b + 1) * LQ, :],
                        dtq_t[:, NB * LD + b * LQ: NB * LD + (b + 1) * LQ],
                        dtq_t[:, b * LD:(b + 1) * LD],
                        start=True, stop=True,
                        tile_position=(0, b * LQ),
                    )

                # kd max-tree: 3 ops, all 2D-contiguous
                tA = sb_wk.tile([P, 2 * LD], f16, name="tA")
                nc.vector.tensor_tensor(tA[:], d2s[:, 0:2 * LD],
                                        d2s[:, 2 * LD:4 * LD], Alu.max)
                dmx = sb_wk.tile([P, LD], f16, name="dmx")
                nc.vector.tensor_tensor(dmx[:], tA[:, 0:LD], tA[:, LD:2 * LD],
                                        Alu.max)
                nc.vector.tensor_tensor(dmx[:], dmx[:], d2s[:, 4 * LD:5 * LD],
                                        Alu.max)

                prd = sb_wk.tile([P, LD], f16, name="prd")
                nc.vector.tensor_tensor(prd[:], s_ps[:], dmx[:], Alu.mult)
                nc.vector.reduce_max(mxall[:, g:g + 1], prd[:],
                                     axis=mybir.AxisListType.X)

            # res = max(mxall, 0), batched (carries the 2^8 factor; the
            # epilogue one-hot matmul undoes it with 2^-8)
            nc.vector.tensor_scalar(rescp[:, 0:G], mxall[:], 0.0, None, Alu.max)

            # ---- merged epilogue: [e4s|ones].T @ [res|cp] ----
            out_ps = ps_s.tile([NB + 1, G + BPC], f32, name="out_ps", tag="spool")
            nc.tensor.matmul(out_ps[:], epi_t[:, 2 * BPC:2 * BPC + NB + 1],
                             rescp[:], start=True, stop=True)
            out_sb = sb_res.tile([NB + 1, G + BPC], f32)
            nc.vector.tensor_copy(out_sb[:], out_ps[:])
            nc.sync.dma_start(out_d[:], out_sb[:])

    nc.compile()
    return nc


def _prep_core_inputs(c, q_repr, q_w, q_ids, q_cls, d_repr, d_w, d_ids, d_cls):
    """Pure layout/packing for one core's 64 pairs."""
    s = slice(c * BPC, (c + 1) * BPC)
    qr = q_repr[s]          # [64, 32, 128] f32
    qw = q_w[s, :, 0]       # [64, 32]
    qi = q_ids[s, :, 0]     # [64, 32] int64
    qc = q_cls[s]           # [64, 128]
    dr = d_repr[s]          # [64, 512, 128]
    dw = d_w[s]             # [64, 512, 5]
    di = d_ids[s]           # [64, 512, 5]
    dc = d_cls[s]           # [64, 128]

    # dtq: [G, D, NB*LD + NB*LQ]: dT cols then qTx cols (qw folded into qT)
    dtq = np.empty((G, D, NB * LD + NB * LQ), np.float16)
    dtq[:, :, :NB * LD] = (
        dr.reshape(G, NB, LD, D).transpose(0, 3, 1, 2).reshape(G, D, NB * LD)
    ).astype(np.float16)
    qrw = qr * qw[:, :, None]   # fold qw into the query reprs
    dtq[:, :, NB * LD:] = (
        qrw.reshape(G, NB, LQ, D).transpose(0, 3, 1, 2).reshape(G, D, NB * LQ)
    ).astype(np.float16)

    # 5-bit id chunks (ids < 2^15)
    qa = (qi >> 10).astype(np.float32)
    qb = ((qi >> 5) & 31).astype(np.float32)
    qcq = (qi & 31).astype(np.float32)
    da = (di >> 10).astype(np.float32)
    db = ((di >> 5) & 31).astype(np.float32)
    dcc = (di & 31).astype(np.float32)
    dw16 = dw.astype(np.float16).astype(np.float32)

    E = np.zeros((NB, P), np.float32)
    for b in range(NB):
        E[b, b * LQ:(b + 1) * LQ] = 1.0

    def dcols(x):
        # [G*NB, LD, KD] batch values -> [G, NB, JD] in l-chunk-major
        # column order: j = lc*(KD*LDC) + kd*LDC + (l - lc*LDC)
        return (x.reshape(G, NB, 2, LDC, KD)
                 .transpose(0, 1, 2, 4, 3).reshape(G, NB, JD))

    # full rhs [G, 26, JD] (l-chunk-major cols) + lhsT [G, 26, 128]
    rhs = np.zeros((G, KSTACK, JD), np.float32)
    rhs[:, 0:4] = dcols(da * da + db * db)
    rhs[:, 4:8] = dcols(dcc * dcc)
    rhs[:, 8:12] = dcols(da)
    rhs[:, 12:16] = dcols(db)
    rhs[:, 16:20] = dcols(dcc)
    rhs[:, 20] = 1.0
    rhs[:, 21] = 1.0
    rhs[:, 22:26] = dcols(dw16)

    qar = qa.reshape(G, P)
    qbr = qb.reshape(G, P)
    qcr = qcq.reshape(G, P)
    lhsT = np.zeros((G, KSTACK, P), np.float32)
    lhsT[:, 0:4] = -E
    lhsT[:, 4:8] = -E
    lhsT[:, 8:12] = (2.0 * qar)[:, None, :] * E
    lhsT[:, 12:16] = (2.0 * qbr)[:, None, :] * E
    lhsT[:, 16:20] = (2.0 * qcr)[:, None, :] * E
    lhsT[:, 20] = -(qar * qar + qbr * qbr)
    lhsT[:, 21] = -(qcr * qcr)
    lhsT[:, 22:26] = EPS * E

    # rhx: [G, 2, 26, 128 + JC]: row-block r = l-chunk r's rhs + lhsT copy
    rhx = np.empty((G, 2, KSTACK, 128 + JC), np.float32)
    for r in range(2):
        rhx[:, r, :, 0:P] = lhsT
        rhx[:, r, :, 128:] = rhs[:, :, r * JC:(r + 1) * JC]

    epi = np.zeros((P, 2 * BPC + NB + 1), np.float32)
    epi[:, 0:BPC] = qc.T
    epi[:, BPC:2 * BPC] = dc.T
    for b in range(NB):
        epi[b * LQ:(b + 1) * LQ, 2 * BPC + b] = 2.0 ** -8
    epi[:, 2 * BPC + NB] = 1.0

    return {
        "dtq": dtq,
        "rhx": rhx.astype(np.float16),
        "epi": epi,
    }


def kernel(q_expert_repr, q_expert_weights, q_expert_ids, q_cls_repr,
           d_expert_repr, d_expert_weights, d_expert_ids, d_cls_repr):
    from concourse.bass_utils import run_bass_kernel_spmd

    q_repr = np.asarray(q_expert_repr, np.float32)
    q_w = np.asarray(q_expert_weights, np.float32)
    q_ids = np.asarray(q_expert_ids, np.int64)
    q_cls = np.asarray(q_cls_repr, np.float32)
    d_repr = np.asarray(d_expert_repr, np.float32)
    d_w = np.asarray(d_expert_weights, np.float32)
    d_ids = np.asarray(d_expert_ids, np.int64)
    d_cls = np.asarray(d_cls_repr, np.float32)

    if "nc" not in _CACHED:
        _CACHED["nc"] = _build_module()
    nc = _CACHED["nc"]

    in_maps = [
        _prep_core_inputs(c, q_repr, q_w, q_ids, q_cls, d_repr, d_w, d_ids, d_cls)
        for c in range(NCORES)
    ]
    rr = run_bass_kernel_spmd(nc, in_maps, core_ids=list(range(NCORES)))

    out = np.zeros((B,), np.float32)
    for c in range(NCORES):
        o = rr.results[c]["out"]            # [NB+1, G+BPC]
        tok = o[0:NB, 0:G]                  # [NB, G]
        cls = o[NB, G:]                     # [BPC]
        out[c * BPC:(c + 1) * BPC] = tok.T.reshape(-1) + cls
    return out


if __name__ == "__main__":
    rng = np.random.default_rng(0)
    ins = {
        "q_expert_repr": rng.standard_normal((B, LQ, D)).astype(np.float32),
        "q_expert_weights": rng.random((B, LQ, KQ)).astype(np.float32),
        "q_expert_ids": rng.integers(0, 30522, (B, LQ, KQ)).astype(np.int64),
        "q_cls_repr": rng.standard_normal((B, D)).astype(np.float32),
        "d_expert_repr": rng.standard_normal((B, LD, D)).astype(np.float32),
        "d_expert_weights": rng.random((B, LD, KD)).astype(np.float32),
        "d_expert_ids": rng.integers(0, 30522, (B, LD, KD)).astype(np.int64),
        "d_cls_repr": rng.standard_normal((B, D)).astype(np.float32),
    }
    out = kernel(**ins)
    print("kernel out[:8]:", out[:8])


# revision 28
# speedup vs baseline: 1.5272x; 1.0172x over previous
"""Trainium2 Bass kernel for moe_routing (nn_CITADEL_15118284882566).

Math: the reference collapses (qw >= 0; the max rows always contain zeros
from non-matches, so negative branches never survive) to, per pair b:

    out[b] = sum_q qw[b,q] * relu( max_{l,kd} sims[b,q,l] * dw[b,l,kd]
                                   * [d_id[b,l,kd] == q_id[b,q]] )
             + dot(q_cls[b], d_cls[b])

Device strategy (data-parallel over B across 8 cores, 64 pairs/core,
16 groups of 4 pairs; partitions = 4 pairs x 32 queries):

1. DIFF2 = -(d_id - q_id)^2 + 2^-12 * dw via 26-row-stacked f16 matmuls
   (two l-chunks of [128, 5*256] f32 PSUM, double buffered). Ids are split
   into three 5-bit chunks (a,b,c <= 31) so -(d-q)^2 expands into bilinear
   rows whose operands are all fp16-exact integers; PSUM f32 accumulation
   is exact, dw rows accumulate last. Matches give DIFF2 = 2^-12*fp16(dw)
   EXACTLY; non-matches give DIFF2 <= -1 + 2^-12 (strictly negative).
2. ACT relu-copies DIFF2 * 2^20 -> f16 (kd-major d2s layout): matches
   become dw*2^8, non-matches 0. No NaN/inf masking needed anywhere.
3. sims via 4 column-tiled matmuls (contraction over D=128 on partitions);
   qw is pre-folded into qT on the host.
4. kd max-tree (3 DVE ops), prd = sims*dmx, reduce_max over l.
5. Single merged epilogue matmul [e4s|ones].T @ [res|cp] -> tok sums
   (scaled 2^-8) and cls dots in one [5, G+BPC] output, one DMA.

PSUM budget: dfc [128,1280] x2 bufs = 6 banks + sims [128,512] x2 = 8.
"""
import sys

sys.path.insert(0, "/opt/trn_rl_repo")

import numpy as np

B, LQ, LD, KQ, KD, D = 512, 32, 512, 1, 5, 128
NCORES = 8
BPC = B // NCORES          # 64 pairs per core
NB = 4                     # pairs per group
G = BPC // NB              # 16 groups
P = 128
JD = KD * LD               # 2560
LDC = LD // 2              # 256 l per l-chunk
JC = KD * LDC              # 1280 cols per chunk
KSTACK = 26
EPS = 2.0 ** -12
SCALE = 2.0 ** 20

_CACHED = {}

# engine-assignment tuning
# NOTE: Pool (GPSIMD) TENSOR_TENSOR fails the V3 ISA engine check in this
# toolchain — GPSIMD can only do DMA triggering / tensor_copy. Concurrent
# PE row-group matmuls writing the same PSUM bank hang the device.
DEFAULT_OPTS = dict(
    dtq_bufs=3,
    rhx_bufs=4,
    d2s_bufs=2,
    dtq_eng="sync",   # dTq DMA engine: sync | scalar | gpsimd
    rhx_eng="gpsimd", # rhx DMA engine
    rowpack=True,     # 2-way PE row-group packing: row-block = l-chunk, so
                      # concurrent streams write disjoint PSUM tiles/banks
)


def _build_module(**kw):
    opts = dict(DEFAULT_OPTS)
    opts.update(kw)
    import concourse.bacc as bacc
    import concourse.mybir as mybir
    from concourse import tile

    f16 = mybir.dt.float16
    f32 = mybir.dt.float32
    Alu = mybir.AluOpType
    Act = mybir.ActivationFunctionType

    nc = bacc.Bacc("TRN2", target_bir_lowering=False, debug=False)

    def eng(name):
        return {"sync": nc.sync, "scalar": nc.scalar, "gpsimd": nc.gpsimd}[name]

    # fused inputs
    dtq_d = nc.dram_tensor("dtq", [G, D, NB * LD + NB * LQ], f16, kind="ExternalInput")
    rhx_d = nc.dram_tensor("rhx", [G, 2, KSTACK, 128 + JC], f16, kind="ExternalInput")
    epi_d = nc.dram_tensor("epi", [P, 2 * BPC + NB + 1], f32, kind="ExternalInput")

    out_d = nc.dram_tensor("out", [NB + 1, G + BPC], f32, kind="ExternalOutput")

    with tile.TileContext(nc) as tc:
        with (
            tc.tile_pool(name="sb_dtq", bufs=opts["dtq_bufs"]) as sb_dtq,
            tc.tile_pool(name="sb_rhx", bufs=opts["rhx_bufs"]) as sb_rhx,
            tc.tile_pool(name="sb_big", bufs=opts["d2s_bufs"]) as sb_big,
            tc.tile_pool(name="sb_wk", bufs=2) as sb_wk,
            tc.tile_pool(name="sb_res", bufs=1) as sb_res,
            tc.tile_pool(name="ps_diff", bufs=2, space="PSUM") as ps_diff,
            tc.tile_pool(name="ps_s", bufs=2, space="PSUM") as ps_s,
        ):
            # rescp: cols [0, G) = res (per-group relu'd maxes; qw is folded
            # into sims), cols [G, G+BPC) = cp (cls elementwise prods)
            rescp = sb_res.tile([P, G + BPC], f32)
            mxall = sb_res.tile([P, G], f32)
            epi_t = sb_res.tile([P, 2 * BPC + NB + 1], f32)
            nc.sync.dma_start(epi_t[:], epi_d[:])
            # cls products, ready as soon as epi lands
            nc.vector.tensor_tensor(rescp[:, G:], epi_t[:, 0:BPC],
                                    epi_t[:, BPC:2 * BPC], Alu.mult)

            for g in range(G):
                dtq_t = sb_dtq.tile([D, NB * LD + NB * LQ], f16, name="dtq_t")
                rhx_t = sb_rhx.tile([58, 128 + JC], f16, name="rhx_t")
                eng(opts["rhx_eng"]).dma_start(rhx_t[0:KSTACK, :], rhx_d[g, 0])
                eng(opts["rhx_eng"]).dma_start(rhx_t[32:32 + KSTACK, :],
                                               rhx_d[g, 1])
                eng(opts["dtq_eng"]).dma_start(dtq_t[:], dtq_d[g, :, :])

                # d2s column layout is kd-major over full l: col = k*LD + l
                # (l = lc*LDC + j), so every tree slice is 2D-contiguous.
                d2s = sb_big.tile([P, JD], f16, name="d2s")
                d2s_k = d2s.rearrange("p (k j) -> p k j", k=KD)
                # row-block = l-chunk: the two dfc pool slots are disjoint
                # PSUM bank sets, so the two PE row-group streams never
                # write the same bank (same-bank interleave hangs the HW).
                dfc0 = ps_diff.tile([P, JC], f32, name="dfc")
                dfc1 = ps_diff.tile([P, JC], f32, name="dfc")
                for k in range(KD):
                    for lc, dfc in ((0, dfc0), (1, dfc1)):
                        base = 32 * lc if opts["rowpack"] else 0
                        nc.tensor.matmul(
                            dfc[:, k * LDC:(k + 1) * LDC],
                            rhx_t[base:base + KSTACK, 0:P],
                            rhx_t[base:base + KSTACK,
                                  128 + k * LDC: 128 + (k + 1) * LDC],
                            start=True, stop=True,
                        )
                for lc, dfc in ((0, dfc0), (1, dfc1)):
                    nc.scalar.activation(
                        d2s_k[:, :, lc * LDC:(lc + 1) * LDC],
                        dfc.rearrange("p (k j) -> p k j", k=KD),
                        Act.Relu, bias=0.0, scale=SCALE)

                s_ps = ps_s.tile([P, LD], f32, name="s_ps", tag="spool")
                for b in range(NB):
                    nc.tensor.matmul(
                        s_ps[b * LQ:(b + 1) * LQ, :],
                        dtq_t[:, NB * LD + b * LQ: NB * LD + (b + 1) * LQ],
                        dtq_t[:, b * LD:(b + 1) * LD],
                        start=True, stop=True,
                        tile_position=(0, b * LQ),
                    )

                # kd max-tree: 3 ops, all 2D-contiguous
                tA = sb_wk.tile([P, 2 * LD], f16, name="tA")
                nc.vector.tensor_tensor(tA[:], d2s[:, 0:2 * LD],
                                        d2s[:, 2 * LD:4 * LD], Alu.max)
                dmx = sb_wk.tile([P, LD], f16, name="dmx")
                nc.vector.tensor_tensor(dmx[:], tA[:, 0:LD], tA[:, LD:2 * LD],
                                        Alu.max)
                nc.vector.tensor_tensor(dmx[:], dmx[:], d2s[:, 4 * LD:5 * LD],
                                        Alu.max)

                prd = sb_wk.tile([P, LD], f16, name="prd")
                nc.vector.tensor_tensor(prd[:], s_ps[:], dmx[:], Alu.mult)
                nc.vector.reduce_max(mxall[:, g:g + 1], prd[:],
                                     axis=mybir.AxisListType.X)

            # res = max(mxall, 0), batched (carries the 2^8 factor; the
            # epilogue one-hot matmul undoes it with 2^-8)
            nc.vector.tensor_scalar(rescp[:, 0:G], mxall[:], 0.0, None, Alu.max)

            # ---- merged epilogue: [e4s|ones].T @ [res|cp] ----
            out_ps = ps_s.tile([NB + 1, G + BPC], f32, name="out_ps", tag="spool")
            nc.tensor.matmul(out_ps[:], epi_t[:, 2 * BPC:2 * BPC + NB + 1],
                             rescp[:], start=True, stop=True)
            out_sb = sb_res.tile([NB + 1, G + BPC], f32)
            nc.vector.tensor_copy(out_sb[:], out_ps[:])
            nc.sync.dma_start(out_d[:], out_sb[:])

    nc.compile()
    return nc


def _prep_core_inputs(c, q_repr, q_w, q_ids, q_cls, d_repr, d_w, d_ids, d_cls):
    """Pure layout/packing for one core's 64 pairs."""
    s = slice(c * BPC, (c + 1) * BPC)
    qr = q_repr[s]          # [64, 32, 128] f32
    qw = q_w[s, :, 0]       # [64, 32]
    qi = q_ids[s, :, 0]     # [64, 32] int64
    qc = q_cls[s]           # [64, 128]
    dr = d_repr[s]          # [64, 512, 128]
    dw = d_w[s]             # [64, 512, 5]
    di = d_ids[s]           # [64, 512, 5]
    dc = d_cls[s]           # [64, 128]

    # dtq: [G, D, NB*LD + NB*LQ]: dT cols then qTx cols (qw folded into qT)
    dtq = np.empty((G, D, NB * LD + NB * LQ), np.float16)
    dtq[:, :, :NB * LD] = (
        dr.reshape(G, NB, LD, D).transpose(0, 3, 1, 2).reshape(G, D, NB * LD)
    ).astype(np.float16)
    qrw = qr * qw[:, :, None]   # fold qw into the query reprs
    dtq[:, :, NB * LD:] = (
        qrw.reshape(G, NB, LQ, D).transpose(0, 3, 1, 2).reshape(G, D, NB * LQ)
    ).astype(np.float16)

    # 5-bit id chunks (ids < 2^15)
    qa = (qi >> 10).astype(np.float32)
    qb = ((qi >> 5) & 31).astype(np.float32)
    qcq = (qi & 31).astype(np.float32)
    da = (di >> 10).astype(np.float32)
    db = ((di >> 5) & 31).astype(np.float32)
    dcc = (di & 31).astype(np.float32)
    dw16 = dw.astype(np.float16).astype(np.float32)

    E = np.zeros((NB, P), np.float32)
    for b in range(NB):
        E[b, b * LQ:(b + 1) * LQ] = 1.0

    def dcols(x):
        # [G*NB, LD, KD] batch values -> [G, NB, JD] in l-chunk-major
        # column order: j = lc*(KD*LDC) + kd*LDC + (l - lc*LDC)
        return (x.reshape(G, NB, 2, LDC, KD)
                 .transpose(0, 1, 2, 4, 3).reshape(G, NB, JD))

    # full rhs [G, 26, JD] (l-chunk-major cols) + lhsT [G, 26, 128]
    rhs = np.zeros((G, KSTACK, JD), np.float32)
    rhs[:, 0:4] = dcols(da * da + db * db)
    rhs[:, 4:8] = dcols(dcc * dcc)
    rhs[:, 8:12] = dcols(da)
    rhs[:, 12:16] = dcols(db)
    rhs[:, 16:20] = dcols(dcc)
    rhs[:, 20] = 1.0
    rhs[:, 21] = 1.0
    rhs[:, 22:26] = dcols(dw16)

    qar = qa.reshape(G, P)
    qbr = qb.reshape(G, P)
    qcr = qcq.reshape(G, P)
    lhsT = np.zeros((G, KSTACK, P), np.float32)
    lhsT[:, 0:4] = -E
    lhsT[:, 4:8] = -E
    lhsT[:, 8:12] = (2.0 * qar)[:, None, :] * E
    lhsT[:, 12:16] = (2.0 * qbr)[:, None, :] * E
    lhsT[:, 16:20] = (2.0 * qcr)[:, None, :] * E
    lhsT[:, 20] = -(qar * qar + qbr * qbr)
    lhsT[:, 21] = -(qcr * qcr)
    lhsT[:, 22:26] = EPS * E

    # rhx: [G, 2, 26, 128 + JC]: row-block r = l-chunk r's rhs + lhsT copy
    rhx = np.empty((G, 2, KSTACK, 128 + JC), np.float32)
    for r in range(2):
        rhx[:, r, :, 0:P] = lhsT
        rhx[:, r, :, 128:] = rhs[:, :, r * JC:(r + 1) * JC]

    epi = np.zeros((P, 2 * BPC + NB + 1), np.float32)
    epi[:, 0:BPC] = qc.T
    epi[:, BPC:2 * BPC] = dc.T
    for b in range(NB):
        epi[b * LQ:(b + 1) * LQ, 2 * BPC + b] = 2.0 ** -8
    epi[:, 2 * BPC + NB] = 1.0

    return {
        "dtq": dtq,
        "rhx": rhx.astype(np.float16),
        "epi": epi,
    }


def kernel(q_expert_repr, q_expert_weights, q_expert_ids, q_cls_repr,
           d_expert_repr, d_expert_weights, d_expert_ids, d_cls_repr):
    from concourse.bass_utils import run_bass_kernel_spmd

    q_repr = np.asarray(q_expert_repr, np.float32)
    q_w = np.asarray(q_expert_weights, np.float32)
    q_ids = np.asarray(q_expert_ids, np.int64)
    q_cls = np.asarray(q_cls_repr, np.float32)
    d_repr = np.asarray(d_expert_repr, np.float32)
    d_w = np.asarray(d_expert_weights, np.float32)
    d_ids = np.asarray(d_expert_ids, np.int64)
    d_cls = np.asarray(d_cls_repr, np.float32)

    if "nc" not in _CACHED:
        _CACHED["nc"] = _build_module()
    nc = _CACHED["nc"]

    in_maps = [
        _prep_core_inputs(c, q_repr, q_w, q_ids, q_cls, d_repr, d_w, d_ids, d_cls)
        for c in range(NCORES)
    ]
    rr = run_bass_kernel_spmd(nc, in_maps, core_ids=list(range(NCORES)))

    out = np.zeros((B,), np.float32)
    for c in range(NCORES):
        o = rr.results[c]["out"]            # [NB+1, G+BPC]
        tok = o[0:NB, 0:G]                  # [NB, G]
        cls = o[NB, G:]                     # [BPC]
        out[c * BPC:(c + 1) * BPC] = tok.T.reshape(-1) + cls
    return out


if __name__ == "__main__":
    rng = np.random.default_rng(0)
    ins = {
        "q_expert_repr": rng.standard_normal((B, LQ, D)).astype(np.float32),
        "q_expert_weights": rng.random((B, LQ, KQ)).astype(np.float32),
        "q_expert_ids": rng.integers(0, 30522, (B, LQ, KQ)).astype(np.int64),
        "q_cls_repr": rng.standard_normal((B, D)).astype(np.float32),
        "d_expert_repr": rng.standard_normal((B, LD, D)).astype(np.float32),
        "d_expert_weights": rng.random((B, LD, KD)).astype(np.float32),
        "d_expert_ids": rng.integers(0, 30522, (B, LD, KD)).astype(np.int64),
        "d_cls_repr": rng.standard_normal((B, D)).astype(np.float32),
    }
    out = kernel(**ins)
    print("kernel out[:8]:", out[:8])
